# revision 6
# baseline (speedup 1.0000x reference)
"""CfC Liquid Cell kernel for Trainium2 (Bass/Tile), 8 NeuronCores.

Sharding: data-parallel over batch (B=8 -> 1 batch element per core).

Single fused loop over time chunks of T=256 (8 chunks), per core:
  - x is pre-transposed, pre-chunked (NB,P,8,T) and cast to bf16 on the
    host -> contiguous 4KB-per-partition chunk DMAs, no PE transposes
  - in_proj matmuls (bf16 weights stationary, x^T moving) -> xz^T in PSUM;
    weights live in 8 per-kt SBUF tiles so the first matmul only waits on
    the first 512KB of weight DMA
  - x_path half cast to SBUF by ACT (with causal halo), z half silu'd
  - depthwise causal conv runs OFF the PE: 4-tap FIR as tensor_tensor
    multiplies with host-broadcast weight tiles; tap0 on GpSimd, taps
    1-3 + adds on DVE (all 2x bf16 mode)
  - head matmuls (bb/f1/f2/tau/decay/state_out) with 2-head block-diagonal
    64x64 weights -> full 128-partition tiles; PSUM tiles are [P,4,T]
    (2 banks) so activations drain 4 matmul outputs per instruction
  - sigmoid(u) = 0.5 + 0.5*tanh(u/2) via 0.5-scaled tau/decay weights; all
    biases in this model are structurally zero (asserted host-side) so
    activations carry no bias and batch freely
  - the scan carries H = 4*h (state_out weights pre-scaled by 0.25):
       r2 = f2 - f1;  s2 = f2 + f1;  q = Tt*r2;  cand2 = s2 + q  # = 2*cand
       dd = 0.5*Td + 0.5;  cpn = (Td-1)*cand2                    # = -4(1-dd)c
       H_t = dd*H_{t-1} - cpn
  - state_out PSUM * silu(z) gating on DVE right after the conv chain
  - out_proj/state_out of previous chunks interleave into the PE stream so
    the PE stays busy while ACT drains gate PSUM; y stores issue on the
    sync queue
"""

import numpy as np

B, S, H = 8, 2048, 1024
NH, HD, NS, K = 16, 64, 64, 4
N_CORES = 8
T = 256             # time chunk
NB = S // T         # 8
P = 128

_CACHE = {}


def _build_program():
    import concourse.bacc as bacc
    import concourse.mybir as mybir
    import concourse.tile as tile

    F32 = mybir.dt.float32
    BF16 = mybir.dt.bfloat16
    AF = mybir.ActivationFunctionType
    ALU = mybir.AluOpType

    nc = bacc.Bacc("TRN2", target_bir_lowering=False, debug=False)

    xT_d = nc.dram_tensor("xT", (NB, P, 8, T), BF16, kind="ExternalInput").ap()
    w_in_d = nc.dram_tensor("w_in", (P, 8, 2 * H), BF16, kind="ExternalInput").ap()
    wt_d = nc.dram_tensor("wt", (P, K, 8, T), BF16, kind="ExternalInput").ap()
    blk_d = nc.dram_tensor("blk", (P, 6, P), BF16, kind="ExternalInput").ap()
    w_out_d = nc.dram_tensor("w_out", (P, 8, H), BF16, kind="ExternalInput").ap()
    y_d = nc.dram_tensor("y", (S, H), F32, kind="ExternalOutput").ap()

    with tile.TileContext(nc) as tc:
        with tc.tile_pool(name="const", bufs=1) as cpool, \
             tc.tile_pool(name="pxT", bufs=2) as pxT:
            # Startup DMA order: first-needed first. xT chunk0 + w_in[0]
            # gate the first matmul; conv weight tiles and blk arrive while
            # chunk-0 in_proj runs; w_out is needed only at chunk 2.
            w_in = [cpool.tile([P, 2 * H], BF16, name=f"w_in{k}") for k in range(8)]
            xT_first = pxT.tile([P, 8, T], BF16, tag="xT", name="xT_first")
            nc.sync.dma_start(xT_first[:], xT_d[0])
            nc.scalar.dma_start(w_in[0][:], w_in_d[:, 0, :])
            nc.sync.dma_start(w_in[1][:], w_in_d[:, 1, :])
            nc.scalar.dma_start(w_in[2][:], w_in_d[:, 2, :])
            nc.sync.dma_start(w_in[3][:], w_in_d[:, 3, :])
            wt = cpool.tile([P, K, 8, T], BF16)
            nc.scalar.dma_start(wt[:, 0:2], wt_d[:, 0:2])
            nc.sync.dma_start(wt[:, 2:4], wt_d[:, 2:4])
            blk = cpool.tile([P, 6, P], BF16)
            nc.scalar.dma_start(blk[:], blk_d[:])
            nc.sync.dma_start(w_in[4][:], w_in_d[:, 4, :])
            nc.scalar.dma_start(w_in[5][:], w_in_d[:, 5, :])
            nc.sync.dma_start(w_in[6][:], w_in_d[:, 6, :])
            nc.scalar.dma_start(w_in[7][:], w_in_d[:, 7, :])
            w_out = cpool.tile([P, 8, H], BF16)
            nc.sync.dma_start(w_out[:, 0:4, :], w_out_d[:, 0:4, :])
            nc.scalar.dma_start(w_out[:, 4:8, :], w_out_d[:, 4:8, :])

            with \
                 tc.tile_pool(name="pxp", bufs=2) as pxp, \
                 tc.tile_pool(name="pzs", bufs=2) as pzs, \
                 tc.tile_pool(name="pcv", bufs=7) as pcv, \
                 tc.tile_pool(name="pxh", bufs=2) as pxh, \
                 tc.tile_pool(name="pbb", bufs=2) as pbb, \
                 tc.tile_pool(name="pgt", bufs=4) as pgt, \
                 tc.tile_pool(name="palg", bufs=1) as palg, \
                 tc.tile_pool(name="pH", bufs=2) as pH, \
                 tc.tile_pool(name="pgh", bufs=2) as pgh, \
                 tc.tile_pool(name="pysb", bufs=3) as pysb, \
                 tc.tile_pool(name="psA", bufs=3, space="PSUM") as psA, \
                 tc.tile_pool(name="psY", bufs=2, space="PSUM") as psY:

                def emit_so(Hk):
                    """state_out matmuls -> two [P,4,T] psum tiles"""
                    tiles = []
                    for half in range(2):
                        ps = psA.tile([P, 4, T], F32, tag="mm", name="so")
                        for sub in range(4):
                            ct = 4 * half + sub
                            nc.tensor.matmul(
                                ps[:, sub, :], blk[:, 5, :], Hk[:, ct, :],
                                start=True, stop=True)
                        tiles.append(ps)
                    return tiles

                def emit_gh(so_tiles, zsk):
                    """gh = state_out_psum * silu(z) (DVE, after conv chain)"""
                    gh = pgh.tile([P, 8, T], BF16, tag="gh", name="gh")
                    for half in range(2):
                        sl = slice(4 * half, 4 * half + 4)
                        nc.vector.tensor_tensor(
                            gh[:, sl, :], so_tiles[half][:], zsk[:, sl, :],
                            ALU.mult)
                    return gh

                def emit_outproj_piece(ghk, piece):
                    """one eighth-chunk of out_proj: 8 accumulating matmuls"""
                    st, hf = piece // 2, piece % 2
                    py = psY.tile([P, 512], F32, tag="y", name="py")
                    for ct in range(8):
                        lh = ghk[:, ct, st * P:(st + 1) * P]
                        nc.tensor.matmul(
                            py[:], lh, w_out[:, ct, hf * 512:(hf + 1) * 512],
                            start=(ct == 0), stop=(ct == 7))
                    return py

                def emit_ystore(c, st, pyA, pyB):
                    ysb = pysb.tile([P, H], F32, tag="ysb", name="ysb")
                    nc.scalar.activation(ysb[:, 0:512], pyA[:], AF.Copy)
                    nc.vector.tensor_copy(ysb[:, 512:1024], pyB[:])
                    r0 = c * T + st * P
                    nc.sync.dma_start(y_d[r0:r0 + P, :], ysb[:])

                xp_prev = None
                H_prev = None
                so_pend = None   # zs tile awaiting state_out of prev chunk
                op_pend = None   # (c-2, gh) awaiting out_proj
                for c in range(NB):
                    if c == 0:
                        xT = xT_first
                    else:
                        xT = pxT.tile([P, 8, T], BF16, tag="xT", name="xT")
                        nc.sync.dma_start(xT[:], xT_d[c])

                    # x_path buffer with 3-column causal halo
                    xp = pxp.tile([P, 8, 3 + T], BF16, tag="xp", name="xp")
                    if c == 0:
                        nc.vector.memset(xp[:, :, :3], 0.0)
                    else:
                        nc.vector.tensor_copy(xp[:, :, :3], xp_prev[:, :, T:T + 3])
                    zs = pzs.tile([P, 8, T], BF16, tag="zs", name="zs")

                    # ---- in_proj: 4 groups of 32 matmuls -> [P,4,T] psum ----
                    for jq in range(4):
                        pm = psA.tile([P, 4, T], F32, tag="mm", name="pm")
                        for sub in range(4):
                            jt = 4 * jq + sub
                            for kt in range(8):
                                nc.tensor.matmul(
                                    pm[:, sub, :],
                                    w_in[kt][:, jt * P:(jt + 1) * P],
                                    xT[:, kt, :],
                                    start=(kt == 0), stop=(kt == 7))
                        if jq < 2:
                            nc.scalar.activation(
                                xp[:, 4 * jq:4 * jq + 4, 3:], pm[:], AF.Copy)
                        else:
                            nc.scalar.activation(
                                zs[:, 4 * (jq - 2):4 * (jq - 2) + 4, :], pm[:],
                                AF.Silu)

                    # ---- depthwise causal conv on DVE/GpSimd (zero bias) ----
                    # y[t] = sum_k w_k * xp[t-3+k]; tap k reads xp[:,:,k:k+T]
                    m0 = pcv.tile([P, 8, T], BF16, tag="cv", name="m0")
                    m1 = pcv.tile([P, 8, T], BF16, tag="cv", name="m1")
                    m2 = pcv.tile([P, 8, T], BF16, tag="cv", name="m2")
                    m3 = pcv.tile([P, 8, T], BF16, tag="cv", name="m3")
                    b23 = pcv.tile([P, 8, T], BF16, tag="cv", name="b23")
                    a01 = pcv.tile([P, 8, T], BF16, tag="cv", name="a01")
                    xhp = pcv.tile([P, 8, T], BF16, tag="cv", name="xhp")
                    nc.gpsimd.tensor_tensor(
                        m0[:], xp[:, :, 0:T], wt[:, 0], ALU.mult)
                    nc.vector.tensor_tensor(
                        m1[:], xp[:, :, 1:1 + T], wt[:, 1], ALU.mult)
                    nc.vector.tensor_tensor(
                        m2[:], xp[:, :, 2:2 + T], wt[:, 2], ALU.mult)
                    nc.vector.tensor_tensor(
                        m3[:], xp[:, :, 3:3 + T], wt[:, 3], ALU.mult)
                    nc.vector.tensor_tensor(b23[:], m2[:], m3[:], ALU.add)
                    nc.vector.tensor_tensor(a01[:], m0[:], m1[:], ALU.add)
                    nc.vector.tensor_tensor(xhp[:], a01[:], b23[:], ALU.add)
                    xh = pxh.tile([P, 8, T], BF16, tag="xh", name="xh")
                    nc.scalar.activation(xh[:, 0:4, :], xhp[:, 0:4, :], AF.Silu)
                    nc.scalar.activation(xh[:, 4:8, :], xhp[:, 4:8, :], AF.Silu)

                    # ---- state_out + gh of previous chunk ----
                    if so_pend is not None:
                        so_tiles = emit_so(H_prev)
                        gh_next = emit_gh(so_tiles, so_pend)
                    else:
                        gh_next = None

                    # ---- out_proj(c-2) first half keeps PE busy while the
                    #      conv chain produces xh ----
                    py_half = [None]
                    if op_pend is not None:
                        pc_, ghk = op_pend
                        py_half[0] = emit_outproj_piece(ghk, 0)
                        py = emit_outproj_piece(ghk, 1)
                        emit_ystore(pc_, 0, py_half[0], py)

                    # ---- backbone ----
                    bbt = pbb.tile([P, 8, T], BF16, tag="bbt", name="bbt")
                    for half in range(2):
                        pb = psA.tile([P, 4, T], F32, tag="mm", name="pb")
                        for sub in range(4):
                            ct = 4 * half + sub
                            nc.tensor.matmul(
                                pb[:, sub, :], blk[:, 0, :], xh[:, ct, :],
                                start=True, stop=True)
                        nc.scalar.activation(
                            bbt[:, 4 * half:4 * half + 4, :], pb[:], AF.Silu)

                    # ---- gate matmuls + tanh + algebra + scans,
                    #      interleaved with out_proj(c-2) second half ----
                    r2 = palg.tile([P, 8, T], BF16, tag="r2", name="r2")
                    s2 = palg.tile([P, 8, T], BF16, tag="s2", name="s2")
                    q = palg.tile([P, 8, T], BF16, tag="q", name="q")
                    cand2 = palg.tile([P, 8, T], BF16, tag="cand2", name="cand2")
                    ddt = palg.tile([P, 8, T], BF16, tag="ddt", name="ddt")
                    Tm1 = palg.tile([P, 8, T], BF16, tag="Tm1", name="Tm1")
                    cpn = palg.tile([P, 8, T], BF16, tag="cpn", name="cpn")
                    Ht = pH.tile([P, 8, T], BF16, tag="H", name="Ht")
                    for cp in range(4):
                        sl = slice(2 * cp, 2 * cp + 2)
                        # F tile: [f1 h0, f1 h1, f2 h0, f2 h1]
                        pF = psA.tile([P, 4, T], F32, tag="mm", name="pF")
                        for g, wi in ((0, 1), (1, 2)):
                            for hh in range(2):
                                nc.tensor.matmul(
                                    pF[:, 2 * g + hh, :], blk[:, wi, :],
                                    bbt[:, 2 * cp + hh, :],
                                    start=True, stop=True)
                        gF = pgt.tile([P, 4, T], BF16, tag="gF", name="gF")
                        nc.scalar.activation(gF[:], pF[:], AF.Tanh)
                        # TD tile: [tau h0, tau h1, dec h0, dec h1]
                        pTD = psA.tile([P, 4, T], F32, tag="mm", name="pTD")
                        for g, wi in ((0, 3), (1, 4)):
                            for hh in range(2):
                                nc.tensor.matmul(
                                    pTD[:, 2 * g + hh, :], blk[:, wi, :],
                                    bbt[:, 2 * cp + hh, :],
                                    start=True, stop=True)
                        gTD = pgt.tile([P, 4, T], BF16, tag="gTD", name="gTD")
                        nc.scalar.activation(gTD[:], pTD[:], AF.Tanh)

                        # out_proj(c-2) pieces 2,3 land mid-gate-phase
                        if op_pend is not None and cp in (1, 2):
                            pc_, ghk = op_pend
                            py = emit_outproj_piece(ghk, cp + 1)
                            if cp == 1:
                                py_half[0] = py
                            else:
                                emit_ystore(pc_, 1, py_half[0], py)

                        # algebra (DVE, 2x bf16) + scans
                        f1a, f2a = gF[:, 0:2, :], gF[:, 2:4, :]
                        tta, tda = gTD[:, 0:2, :], gTD[:, 2:4, :]
                        nc.vector.tensor_tensor(
                            r2[:, sl, :], f2a, f1a, ALU.subtract)
                        nc.vector.tensor_tensor(
                            s2[:, sl, :], f2a, f1a, ALU.add)
                        nc.vector.tensor_tensor(
                            q[:, sl, :], tta, r2[:, sl, :], ALU.mult)
                        nc.vector.tensor_tensor(
                            cand2[:, sl, :], s2[:, sl, :], q[:, sl, :], ALU.add)
                        nc.vector.tensor_scalar(
                            ddt[:, sl, :], tda, 0.5, 0.5, ALU.mult, ALU.add)
                        nc.vector.tensor_scalar_sub(
                            Tm1[:, sl, :], tda, 1.0)
                        nc.vector.tensor_tensor(
                            cpn[:, sl, :], Tm1[:, sl, :], cand2[:, sl, :],
                            ALU.mult)
                        for ct in (2 * cp, 2 * cp + 1):
                            init = 0.0 if c == 0 else H_prev[:, ct, T - 1:T]
                            nc.vector.tensor_tensor_scan(
                                Ht[:, ct, :], ddt[:, ct, :], cpn[:, ct, :], init,
                                ALU.mult, ALU.subtract)

                    xp_prev = xp
                    H_prev = Ht
                    so_pend = zs
                    op_pend = (c - 1, gh_next) if gh_next is not None else None

                # tail: state_out/gh of the final chunk, pending out_proj
                # (NB-2), then out_proj of the final chunk
                so_tiles = emit_so(H_prev)
                gh_last = emit_gh(so_tiles, so_pend)
                if op_pend is not None:
                    pc_, ghk = op_pend
                    for piece in range(4):
                        py = emit_outproj_piece(ghk, piece)
                        if piece % 2 == 0:
                            py_hold = py
                        else:
                            emit_ystore(pc_, piece // 2, py_hold, py)
                for piece in range(4):
                    py = emit_outproj_piece(gh_last, piece)
                    if piece % 2 == 0:
                        py_hold = py
                    else:
                        emit_ystore(NB - 1, piece // 2, py_hold, py)

    nc.compile()
    return nc


def _prep_shared(inputs):
    """Host-side preprocessing of the shared (weight) tensors."""
    import ml_dtypes
    f32 = np.float32
    bf16 = ml_dtypes.bfloat16

    # The kernel drops all bias adds: every bias in this model is zero.
    for bname in ("conv_b", "bb_b", "f1_b", "f2_b", "tau_a_b", "tau_b",
                  "decay_b", "state_out_b"):
        assert not np.any(np.asarray(inputs[bname])), f"nonzero {bname}"

    in_proj_w = np.asarray(inputs["in_proj_w"], f32)
    conv_w = np.asarray(inputs["conv_w"], f32)

    w_in = in_proj_w.reshape(8, P, 2 * H).transpose(1, 0, 2)

    # conv weights broadcast along T: wt[p, k, ct, t] = conv_w[ct*128+p, 0, k]
    cw = conv_w[:, 0, :].reshape(8, P, K).transpose(1, 2, 0)  # (P, K, 8)
    wt = np.broadcast_to(cw[:, :, :, None], (P, K, 8, T))

    w_out = np.asarray(inputs["out_proj_w"], f32).reshape(8, P, H).transpose(1, 0, 2)

    def blk2(w):
        o = np.zeros((P, P), f32)
        o[:64, :64] = w
        o[64:, 64:] = w
        return o

    blk = np.stack([
        blk2(np.asarray(inputs["bb_w"], f32)),
        blk2(np.asarray(inputs["f1_w"], f32)),
        blk2(np.asarray(inputs["f2_w"], f32)),
        blk2(np.asarray(inputs["tau_a_w"], f32) * 0.5),   # sigmoid via tanh
        blk2(np.asarray(inputs["decay_w"], f32) * 0.5),   # sigmoid via tanh
        blk2(np.asarray(inputs["state_out_w"], f32) * 0.25),  # scan carries 4h
    ], axis=1)  # (P, 6, P)

    return {
        "w_in": np.ascontiguousarray(w_in.astype(bf16)),
        "wt": np.ascontiguousarray(wt.astype(bf16)),
        "blk": np.ascontiguousarray(blk.astype(bf16)),
        "w_out": np.ascontiguousarray(w_out.astype(bf16)),
    }


def _make_in_maps(inputs):
    import ml_dtypes

    shared = _prep_shared(inputs)
    x = np.asarray(inputs["x"], np.float32)
    in_maps = []
    for b in range(N_CORES):
        m = dict(shared)
        # (P, 8, S) feature-major, then chunked to (NB, P, 8, T)
        xT = x[b].T.reshape(8, P, S).transpose(1, 0, 2)
        xTc = xT.reshape(P, 8, NB, T).transpose(2, 0, 1, 3)
        m["xT"] = np.ascontiguousarray(xTc.astype(ml_dtypes.bfloat16))
        in_maps.append(m)
    return in_maps


def kernel(**inputs) -> np.ndarray:
    from concourse import bass_utils

    if "nc" not in _CACHE:
        _CACHE["nc"] = _build_program()
    nc = _CACHE["nc"]

    in_maps = _make_in_maps(inputs)
    res = bass_utils.run_bass_kernel_spmd(nc, in_maps, core_ids=list(range(N_CORES)))
    out = np.stack([res.results[b]["y"] for b in range(N_CORES)], axis=0)
    return out.astype(np.float32)


# revision 12
# speedup vs baseline: 1.0101x; 1.0101x over previous
"""CfC Liquid Cell kernel for Trainium2 (Bass/Tile), 8 NeuronCores.

Sharding: data-parallel over batch (B=8 -> 1 batch element per core).

Single fused loop over time chunks of T=256 (8 chunks), per core:
  - x is pre-transposed, pre-chunked (NB,P,8,T) and cast to bf16 on the
    host -> contiguous 4KB-per-partition chunk DMAs, no PE transposes
  - in_proj matmuls (bf16 weights stationary, x^T moving) -> xz^T in PSUM;
    weights live in 8 per-kt SBUF tiles so the first matmul only waits on
    the first 512KB of weight DMA
  - x_path half cast to SBUF by ACT (with causal halo), z half silu'd
  - depthwise causal conv runs OFF the PE: 4-tap FIR as tensor_tensor
    multiplies with host-broadcast weight tiles; tap0 on GpSimd, taps
    1-3 + adds on DVE (all 2x bf16 mode)
  - head matmuls (bb/f1/f2/tau/decay/state_out) with 2-head block-diagonal
    64x64 weights -> full 128-partition tiles; PSUM tiles are [P,4,T]
    (2 banks) so activations drain 4 matmul outputs per instruction
  - sigmoid(u) = 0.5 + 0.5*tanh(u/2) via 0.5-scaled tau/decay weights; all
    biases in this model are structurally zero (asserted host-side) so
    activations carry no bias and batch freely
  - the scan carries H = 4*h (state_out weights pre-scaled by 0.25):
       r2 = f2 - f1;  s2 = f2 + f1;  q = Tt*r2;  cand2 = s2 + q  # = 2*cand
       dd = 0.5*Td + 0.5;  cpn = (Td-1)*cand2                    # = -4(1-dd)c
       H_t = dd*H_{t-1} - cpn
  - state_out PSUM * silu(z) gating on DVE right after the conv chain
  - out_proj/state_out of previous chunks interleave into the PE stream so
    the PE stays busy while ACT drains gate PSUM; y stores issue on the
    sync queue
"""

import numpy as np

B, S, H = 8, 2048, 1024
NH, HD, NS, K = 16, 64, 64, 4
N_CORES = 8
T = 256             # time chunk
NB = S // T         # 8
P = 128

_CACHE = {}


def _build_program():
    import concourse.bacc as bacc
    import concourse.mybir as mybir
    import concourse.tile as tile

    F32 = mybir.dt.float32
    BF16 = mybir.dt.bfloat16
    AF = mybir.ActivationFunctionType
    ALU = mybir.AluOpType

    nc = bacc.Bacc("TRN2", target_bir_lowering=False, debug=False)

    xT_d = nc.dram_tensor("xT", (NB, P, 8, T), BF16, kind="ExternalInput").ap()
    w_in_d = nc.dram_tensor("w_in", (P, 8, 2 * H), BF16, kind="ExternalInput").ap()
    wt_d = nc.dram_tensor("wt", (P, K, 8, T), BF16, kind="ExternalInput").ap()
    blk_d = nc.dram_tensor("blk", (P, 6, P), BF16, kind="ExternalInput").ap()
    w_out_d = nc.dram_tensor("w_out", (P, 8, H), BF16, kind="ExternalInput").ap()
    y_d = nc.dram_tensor("y", (S, H), F32, kind="ExternalOutput").ap()

    with tile.TileContext(nc) as tc:
        with tc.tile_pool(name="const", bufs=1) as cpool, \
             tc.tile_pool(name="pxT", bufs=2) as pxT:
            # Startup DMA order: first-needed first. xT chunk0 + w_in[0]
            # gate the first matmul; conv weight tiles and blk arrive while
            # chunk-0 in_proj runs; w_out is needed only at chunk 2.
            w_in = [cpool.tile([P, 2 * H], BF16, name=f"w_in{k}") for k in range(8)]
            xT_first = pxT.tile([P, 8, T], BF16, tag="xT", name="xT_first")
            nc.sync.dma_start(xT_first[:], xT_d[0])
            nc.scalar.dma_start(w_in[0][:], w_in_d[:, 0, :])
            nc.sync.dma_start(w_in[1][:], w_in_d[:, 1, :])
            nc.scalar.dma_start(w_in[2][:], w_in_d[:, 2, :])
            nc.sync.dma_start(w_in[3][:], w_in_d[:, 3, :])
            wt = cpool.tile([P, K, 8, T], BF16)
            nc.scalar.dma_start(wt[:, 0:2], wt_d[:, 0:2])
            nc.sync.dma_start(wt[:, 2:4], wt_d[:, 2:4])
            blk = cpool.tile([P, 6, P], BF16)
            nc.scalar.dma_start(blk[:], blk_d[:])
            nc.sync.dma_start(w_in[4][:], w_in_d[:, 4, :])
            nc.scalar.dma_start(w_in[5][:], w_in_d[:, 5, :])
            nc.sync.dma_start(w_in[6][:], w_in_d[:, 6, :])
            nc.scalar.dma_start(w_in[7][:], w_in_d[:, 7, :])
            w_out = cpool.tile([P, 8, H], BF16)
            nc.sync.dma_start(w_out[:, 0:4, :], w_out_d[:, 0:4, :])
            nc.scalar.dma_start(w_out[:, 4:8, :], w_out_d[:, 4:8, :])

            with \
                 tc.tile_pool(name="pxp", bufs=2) as pxp, \
                 tc.tile_pool(name="pzs", bufs=3) as pzs, \
                 tc.tile_pool(name="pcv", bufs=7) as pcv, \
                 tc.tile_pool(name="pxh", bufs=2) as pxh, \
                 tc.tile_pool(name="pbb", bufs=2) as pbb, \
                 tc.tile_pool(name="pgt", bufs=4) as pgt, \
                 tc.tile_pool(name="palg", bufs=2) as palg, \
                 tc.tile_pool(name="pH", bufs=2) as pH, \
                 tc.tile_pool(name="pgh", bufs=2) as pgh, \
                 tc.tile_pool(name="pysb", bufs=2) as pysb, \
                 tc.tile_pool(name="psA", bufs=3, space="PSUM") as psA, \
                 tc.tile_pool(name="psY", bufs=2, space="PSUM") as psY:

                def emit_so(Hk):
                    """state_out matmuls -> two [P,4,T] psum tiles"""
                    tiles = []
                    for half in range(2):
                        ps = psA.tile([P, 4, T], F32, tag="mm", name="so")
                        for sub in range(4):
                            ct = 4 * half + sub
                            nc.tensor.matmul(
                                ps[:, sub, :], blk[:, 5, :], Hk[:, ct, :],
                                start=True, stop=True)
                        tiles.append(ps)
                    return tiles

                def emit_gh(so_tiles, zsk):
                    """gh = state_out_psum * silu(z) (DVE, after conv chain)"""
                    gh = pgh.tile([P, 8, T], BF16, tag="gh", name="gh")
                    for half in range(2):
                        sl = slice(4 * half, 4 * half + 4)
                        nc.vector.tensor_tensor(
                            gh[:, sl, :], so_tiles[half][:], zsk[:, sl, :],
                            ALU.mult)
                    return gh

                def emit_outproj_piece(ghk, piece):
                    """one eighth-chunk of out_proj: 8 accumulating matmuls"""
                    st, hf = piece // 2, piece % 2
                    py = psY.tile([P, 512], F32, tag="y", name="py")
                    for ct in range(8):
                        lh = ghk[:, ct, st * P:(st + 1) * P]
                        nc.tensor.matmul(
                            py[:], lh, w_out[:, ct, hf * 512:(hf + 1) * 512],
                            start=(ct == 0), stop=(ct == 7))
                    return py

                def emit_ystore(c, st, pyA, pyB):
                    ysb = pysb.tile([P, H], F32, tag="ysb", name="ysb")
                    nc.scalar.activation(ysb[:, 0:512], pyA[:], AF.Copy)
                    nc.vector.tensor_copy(ysb[:, 512:1024], pyB[:])
                    r0 = c * T + st * P
                    nc.sync.dma_start(y_d[r0:r0 + P, :], ysb[:])

                def emit_op_batch(c, ghk):
                    for piece in range(4):
                        py = emit_outproj_piece(ghk, piece)
                        if piece % 2 == 0:
                            py_hold = py
                        else:
                            emit_ystore(c, piece // 2, py_hold, py)

                xp_of, zs_of, xh_of, Ht_of, gh_of = {}, {}, {}, {}, {}

                def emit_X(c):
                    """in_proj + conv chain of chunk c (PE/ACT/DVE/Pool)"""
                    if c == 0:
                        xT = xT_first
                    else:
                        xT = pxT.tile([P, 8, T], BF16, tag="xT", name="xT")
                        nc.sync.dma_start(xT[:], xT_d[c])

                    # x_path buffer with 3-column causal halo
                    xp = pxp.tile([P, 8, 3 + T], BF16, tag="xp", name="xp")
                    if c == 0:
                        nc.vector.memset(xp[:, :, :3], 0.0)
                    else:
                        nc.vector.tensor_copy(
                            xp[:, :, :3], xp_of.pop(c - 1)[:, :, T:T + 3])
                    xp_of[c] = xp
                    zs = pzs.tile([P, 8, T], BF16, tag="zs", name="zs")
                    zs_of[c] = zs

                    # in_proj: 4 groups of 32 matmuls -> [P,4,T] psum
                    for jq in range(4):
                        pm = psA.tile([P, 4, T], F32, tag="mm", name="pm")
                        for sub in range(4):
                            jt = 4 * jq + sub
                            for kt in range(8):
                                nc.tensor.matmul(
                                    pm[:, sub, :],
                                    w_in[kt][:, jt * P:(jt + 1) * P],
                                    xT[:, kt, :],
                                    start=(kt == 0), stop=(kt == 7))
                        if jq < 2:
                            nc.scalar.activation(
                                xp[:, 4 * jq:4 * jq + 4, 3:], pm[:], AF.Copy)
                        else:
                            nc.scalar.activation(
                                zs[:, 4 * (jq - 2):4 * (jq - 2) + 4, :], pm[:],
                                AF.Silu)

                    # depthwise causal conv on DVE/GpSimd (zero bias):
                    # y[t] = sum_k w_k * xp[t-3+k]; tap k reads xp[:,:,k:k+T]
                    m0 = pcv.tile([P, 8, T], BF16, tag="cv", name="m0")
                    m1 = pcv.tile([P, 8, T], BF16, tag="cv", name="m1")
                    m2 = pcv.tile([P, 8, T], BF16, tag="cv", name="m2")
                    m3 = pcv.tile([P, 8, T], BF16, tag="cv", name="m3")
                    b23 = pcv.tile([P, 8, T], BF16, tag="cv", name="b23")
                    a01 = pcv.tile([P, 8, T], BF16, tag="cv", name="a01")
                    xhp = pcv.tile([P, 8, T], BF16, tag="cv", name="xhp")
                    nc.gpsimd.tensor_tensor(
                        m0[:], xp[:, :, 0:T], wt[:, 0], ALU.mult)
                    nc.vector.tensor_tensor(
                        m1[:], xp[:, :, 1:1 + T], wt[:, 1], ALU.mult)
                    nc.vector.tensor_tensor(
                        m2[:], xp[:, :, 2:2 + T], wt[:, 2], ALU.mult)
                    nc.vector.tensor_tensor(
                        m3[:], xp[:, :, 3:3 + T], wt[:, 3], ALU.mult)
                    nc.vector.tensor_tensor(b23[:], m2[:], m3[:], ALU.add)
                    nc.vector.tensor_tensor(a01[:], m0[:], m1[:], ALU.add)
                    nc.vector.tensor_tensor(xhp[:], a01[:], b23[:], ALU.add)
                    xh = pxh.tile([P, 8, T], BF16, tag="xh", name="xh")
                    nc.scalar.activation(xh[:, 0:4, :], xhp[:, 0:4, :], AF.Silu)
                    nc.scalar.activation(xh[:, 4:8, :], xhp[:, 4:8, :], AF.Silu)
                    xh_of[c] = xh

                def emit_Y(c):
                    """bb/gates/algebra/scans of chunk c + so(c-1) + op(c-2).

                    Emitted after X(c+1): the conv chain of c+1 hides behind
                    in_proj(c+1) matmuls, and this block's PE work hides the
                    DVE algebra latency of chunk c.
                    """
                    if c >= 1:
                        so_tiles = emit_so(Ht_of[c - 1])
                        gh_of[c - 1] = emit_gh(so_tiles, zs_of.pop(c - 1))
                    ghk = gh_of.pop(c - 2, None)
                    py_half = [None]
                    if ghk is not None:
                        py_half[0] = emit_outproj_piece(ghk, 0)
                        py = emit_outproj_piece(ghk, 1)
                        emit_ystore(c - 2, 0, py_half[0], py)

                    # backbone
                    xh = xh_of.pop(c)
                    bbt = pbb.tile([P, 8, T], BF16, tag="bbt", name="bbt")
                    for half in range(2):
                        pb = psA.tile([P, 4, T], F32, tag="mm", name="pb")
                        for sub in range(4):
                            ct = 4 * half + sub
                            nc.tensor.matmul(
                                pb[:, sub, :], blk[:, 0, :], xh[:, ct, :],
                                start=True, stop=True)
                        nc.scalar.activation(
                            bbt[:, 4 * half:4 * half + 4, :], pb[:], AF.Silu)

                    # gates + tanh + algebra + scans, op(c-2) pieces 2,3
                    Ht = pH.tile([P, 8, T], BF16, tag="H", name="Ht")
                    for cp in range(4):
                        # per-cp transient algebra scratch
                        r2 = palg.tile([P, 2, T], BF16, tag="r2", name="r2")
                        s2 = palg.tile([P, 2, T], BF16, tag="s2", name="s2")
                        q = palg.tile([P, 2, T], BF16, tag="q", name="q")
                        cand2 = palg.tile([P, 2, T], BF16, tag="cand2", name="cand2")
                        ddt = palg.tile([P, 2, T], BF16, tag="ddt", name="ddt")
                        Tm1 = palg.tile([P, 2, T], BF16, tag="Tm1", name="Tm1")
                        cpn = palg.tile([P, 2, T], BF16, tag="cpn", name="cpn")
                        # F tile: [f1 h0, f1 h1, f2 h0, f2 h1]
                        pF = psA.tile([P, 4, T], F32, tag="mm", name="pF")
                        for g, wi in ((0, 1), (1, 2)):
                            for hh in range(2):
                                nc.tensor.matmul(
                                    pF[:, 2 * g + hh, :], blk[:, wi, :],
                                    bbt[:, 2 * cp + hh, :],
                                    start=True, stop=True)
                        gF = pgt.tile([P, 4, T], BF16, tag="gF", name="gF")
                        nc.scalar.activation(gF[:], pF[:], AF.Tanh)
                        # TD tile: [tau h0, tau h1, dec h0, dec h1]
                        pTD = psA.tile([P, 4, T], F32, tag="mm", name="pTD")
                        for g, wi in ((0, 3), (1, 4)):
                            for hh in range(2):
                                nc.tensor.matmul(
                                    pTD[:, 2 * g + hh, :], blk[:, wi, :],
                                    bbt[:, 2 * cp + hh, :],
                                    start=True, stop=True)
                        gTD = pgt.tile([P, 4, T], BF16, tag="gTD", name="gTD")
                        nc.scalar.activation(gTD[:], pTD[:], AF.Tanh)

                        if ghk is not None and cp in (1, 2):
                            py = emit_outproj_piece(ghk, cp + 1)
                            if cp == 1:
                                py_half[0] = py
                            else:
                                emit_ystore(c - 2, 1, py_half[0], py)

                        # algebra (DVE, 2x bf16) + scans
                        f1a, f2a = gF[:, 0:2, :], gF[:, 2:4, :]
                        tta, tda = gTD[:, 0:2, :], gTD[:, 2:4, :]
                        nc.vector.tensor_tensor(r2[:], f2a, f1a, ALU.subtract)
                        nc.vector.tensor_tensor(s2[:], f2a, f1a, ALU.add)
                        nc.vector.tensor_tensor(q[:], tta, r2[:], ALU.mult)
                        nc.vector.tensor_tensor(
                            cand2[:], s2[:], q[:], ALU.add)
                        nc.vector.tensor_scalar(
                            ddt[:], tda, 0.5, 0.5, ALU.mult, ALU.add)
                        nc.vector.tensor_scalar_sub(Tm1[:], tda, 1.0)
                        nc.vector.tensor_tensor(
                            cpn[:], Tm1[:], cand2[:], ALU.mult)
                        for i, ct in enumerate((2 * cp, 2 * cp + 1)):
                            init = (0.0 if c == 0
                                    else Ht_of[c - 1][:, ct, T - 1:T])
                            nc.vector.tensor_tensor_scan(
                                Ht[:, ct, :], ddt[:, i, :], cpn[:, i, :], init,
                                ALU.mult, ALU.subtract)
                    Ht_of.pop(c - 1, None)
                    Ht_of[c] = Ht

                    # last Y: drain gh(c-1) here so only one out_proj
                    # remains for the tail
                    if c == NB - 1:
                        emit_op_batch(c - 1, gh_of.pop(c - 1))

                for c in range(NB):
                    emit_X(c)
                    if c >= 1:
                        emit_Y(c - 1)
                emit_Y(NB - 1)

                # tail: state_out/gh/out_proj of the final chunk
                so_tiles = emit_so(Ht_of[NB - 1])
                gh_last = emit_gh(so_tiles, zs_of.pop(NB - 1))
                emit_op_batch(NB - 1, gh_last)

    nc.compile()
    return nc


def _prep_shared(inputs):
    """Host-side preprocessing of the shared (weight) tensors."""
    import ml_dtypes
    f32 = np.float32
    bf16 = ml_dtypes.bfloat16

    # The kernel drops all bias adds: every bias in this model is zero.
    for bname in ("conv_b", "bb_b", "f1_b", "f2_b", "tau_a_b", "tau_b",
                  "decay_b", "state_out_b"):
        assert not np.any(np.asarray(inputs[bname])), f"nonzero {bname}"

    in_proj_w = np.asarray(inputs["in_proj_w"], f32)
    conv_w = np.asarray(inputs["conv_w"], f32)

    w_in = in_proj_w.reshape(8, P, 2 * H).transpose(1, 0, 2)

    # conv weights broadcast along T: wt[p, k, ct, t] = conv_w[ct*128+p, 0, k]
    cw = conv_w[:, 0, :].reshape(8, P, K).transpose(1, 2, 0)  # (P, K, 8)
    wt = np.broadcast_to(cw[:, :, :, None], (P, K, 8, T))

    w_out = np.asarray(inputs["out_proj_w"], f32).reshape(8, P, H).transpose(1, 0, 2)

    def blk2(w):
        o = np.zeros((P, P), f32)
        o[:64, :64] = w
        o[64:, 64:] = w
        return o

    blk = np.stack([
        blk2(np.asarray(inputs["bb_w"], f32)),
        blk2(np.asarray(inputs["f1_w"], f32)),
        blk2(np.asarray(inputs["f2_w"], f32)),
        blk2(np.asarray(inputs["tau_a_w"], f32) * 0.5),   # sigmoid via tanh
        blk2(np.asarray(inputs["decay_w"], f32) * 0.5),   # sigmoid via tanh
        blk2(np.asarray(inputs["state_out_w"], f32) * 0.25),  # scan carries 4h
    ], axis=1)  # (P, 6, P)

    return {
        "w_in": np.ascontiguousarray(w_in.astype(bf16)),
        "wt": np.ascontiguousarray(wt.astype(bf16)),
        "blk": np.ascontiguousarray(blk.astype(bf16)),
        "w_out": np.ascontiguousarray(w_out.astype(bf16)),
    }


def _make_in_maps(inputs):
    import ml_dtypes

    shared = _prep_shared(inputs)
    x = np.asarray(inputs["x"], np.float32)
    in_maps = []
    for b in range(N_CORES):
        m = dict(shared)
        # (P, 8, S) feature-major, then chunked to (NB, P, 8, T)
        xT = x[b].T.reshape(8, P, S).transpose(1, 0, 2)
        xTc = xT.reshape(P, 8, NB, T).transpose(2, 0, 1, 3)
        m["xT"] = np.ascontiguousarray(xTc.astype(ml_dtypes.bfloat16))
        in_maps.append(m)
    return in_maps


def kernel(**inputs) -> np.ndarray:
    from concourse import bass_utils

    if "nc" not in _CACHE:
        _CACHE["nc"] = _build_program()
    nc = _CACHE["nc"]

    in_maps = _make_in_maps(inputs)
    res = bass_utils.run_bass_kernel_spmd(nc, in_maps, core_ids=list(range(N_CORES)))
    out = np.stack([res.results[b]["y"] for b in range(N_CORES)], axis=0)
    return out.astype(np.float32)
